# revision 6
# baseline (speedup 1.0000x reference)
"""Causal multi-head attention (B=4, S=2048, D=1024, H=16) on 8 TRN2 NeuronCores.

Sharding: DP=4 over batch x TP=2 over heads (8 heads per core). Each core:
  - receives transposed activations xT = x[b].T (host-prepared, bf16),
    column shards of Wq/Wk/Wv (512 cols = 8 heads) and the row shard of Wo.
  - computes V (natural layout, with a ones-column per head that yields the
    softmax denominators inside the PV matmul), then per head-pair p:
    KT[p]/QT[p] projections -> scoresT = K_h Q_h^T (2-head row-packed
    matmuls, causal tile skipping) -> probsT = exp(scoresT/8) * causal mask
    -> PV -> numerator^T + denominator -> per-j reciprocal + broadcast
    normalization pipelined behind the next j-tile's compute,
  - finally the partial output A^T.T @ Wo_shard in [seq, D] layout.
  - host sums the two TP partials per batch and adds bo.

Perf structure:
  - input DMAs spread over four engine queues (scalar: weights, gpsimd +
    vector: xv column-chunks, sync: xk then xq) so the first V-proj matmul
    isn't serialized behind 15 MB on one queue;
  - diagonal-crossing score/PV matmuls and mask muls only touch the causally
    valid query columns (matmul cost is per output column);
  - softmax normalization runs per (pair, j-tile): DVE reciprocal of the
    denominator row -> DRAM row -> partition-broadcast DMA -> atp multiply
    on the (otherwise idle) GpSimd engine, so the DVE queue never blocks
    on a DMA round-trip and the O-projection starts as soon as the last
    PV drains.
"""

import sys

sys.path.insert(0, "/opt/trn_rl_repo")

import numpy as np

B = 4
S = 2048
D = 1024
H = 16
HD = 64
TP = 2
DH = D // TP          # 512 head-dims per core (8 heads)
NHL = DH // HD        # 8 local heads
DCH = 4               # dchunks of 128 within DH
NKT = S // 128        # 16 key tiles
NQT = S // 512        # 4 query tiles
KCH = D // 128        # 8 contraction tiles for projections
GRP = 2               # score k-tiles grouped per exp op

_compiled = None


def _build():
    import concourse.bacc as bacc
    import concourse.mybir as mybir
    import concourse.tile as tile

    F32 = mybir.dt.float32
    BF16 = mybir.dt.bfloat16
    EXP = mybir.ActivationFunctionType.Exp

    nc = bacc.Bacc("TRN2", target_bir_lowering=False, debug=False)

    xq = nc.dram_tensor("xq", [D, S], BF16, kind="ExternalInput")
    xk = nc.dram_tensor("xk", [D, S], BF16, kind="ExternalInput")
    xv = nc.dram_tensor("xv", [D, S], BF16, kind="ExternalInput")
    wq = nc.dram_tensor("wq", [D, DH], BF16, kind="ExternalInput")
    wk = nc.dram_tensor("wk", [D, DH], BF16, kind="ExternalInput")
    wv = nc.dram_tensor("wv", [D, DH], BF16, kind="ExternalInput")
    wo = nc.dram_tensor("wo", [DH, D], BF16, kind="ExternalInput")
    bq_c = nc.dram_tensor("bq_c", [128, DCH], F32, kind="ExternalInput")
    bk_c = nc.dram_tensor("bk_c", [128, DCH], F32, kind="ExternalInput")
    bv_b = nc.dram_tensor("bv_b", [128, DH], F32, kind="ExternalInput")
    out = nc.dram_tensor("out", [S, D], F32, kind="ExternalOutput")
    rden = nc.dram_tensor("rden", [NHL, S], F32)    # reciprocals bounce

    with tile.TileContext(nc) as tc:
        with (
            tc.tile_pool(name="qt", bufs=1) as qt_pool,
            tc.tile_pool(name="kt", bufs=1) as kt_pool,
            tc.tile_pool(name="vn", bufs=1) as vn_pool,
            tc.tile_pool(name="cst", bufs=1) as cst,
        ):
            QT = [qt_pool.tile([128, S], BF16, tag=f"qt{d}", name=f"QT{d}")
                  for d in range(DCH)]
            KT = [kt_pool.tile([128, S], BF16, tag=f"kt{d}", name=f"KT{d}")
                  for d in range(DCH)]
            # V natural [seq, 8*(64+1)]: head h cols 65h..65h+63, ones at 65h+64
            VN = [vn_pool.tile([128, NHL * (HD + 1)], BF16, tag=f"vn{i}",
                               name=f"VN{i}")
                  for i in range(NKT)]

            bqs = cst.tile([128, DCH], F32, tag="bqs", name="bqs")
            bks = cst.tile([128, DCH], F32, tag="bks", name="bks")
            bvb = cst.tile([128, DH], F32, tag="bvb", name="bvb")
            nc.scalar.dma_start(out=bqs[:, :], in_=bq_c[:, :])
            nc.scalar.dma_start(out=bks[:, :], in_=bk_c[:, :])
            nc.scalar.dma_start(out=bvb[:, :], in_=bv_b[:, :])

            # causal mask base [128, 896]: mask[x, c] = 1.0 iff c - x >= 384.
            # crossing k-tile i (0..3) of a 512-q tile uses slice
            # mask[:, 384 : 896-128i] over query cols 128i..512
            # ->  valid iff y >= x + 128 i.
            mask = cst.tile([128, 896], BF16, tag="mask", name="mask")
            nc.gpsimd.memset(mask[:, :], 1.0)
            nc.gpsimd.affine_select(
                out=mask[:, :],
                in_=mask[:, :],
                compare_op=mybir.AluOpType.is_ge,
                fill=0.0,
                base=-384,
                pattern=[[1, 896]],
                channel_multiplier=-1,
            )

            ones = cst.tile([128, NHL], F32, tag="ones", name="ones")
            nc.vector.memset(ones[:, :], 1.0)
            for v in VN:
                nc.vector.tensor_copy(v[:, HD::HD + 1], ones[:, :])

            atp_ctx = tc.tile_pool(name="atp", bufs=4)
            atp_pool = atp_ctx.__enter__()
            wo_ctx = tc.tile_pool(name="wo", bufs=1)
            wo_pool = wo_ctx.__enter__()
            ob_ctx = tc.tile_pool(name="ob", bufs=2)
            ob_pool = ob_ctx.__enter__()
            with (
                tc.tile_pool(name="xkq", bufs=1) as xkq_pool,
                tc.tile_pool(name="wp", bufs=1) as wp_pool,
                tc.tile_pool(name="pr", bufs=4) as pr_pool,
                tc.tile_pool(name="nrm", bufs=2) as nrm_pool,
                tc.tile_pool(name="psA", bufs=2, space="PSUM") as psA,
                tc.tile_pool(name="psS", bufs=2, space="PSUM") as psS,
                tc.tile_pool(name="psV", bufs=1, space="PSUM") as psV,
            ):
                wvt = [wp_pool.tile([128, DH], BF16, tag=f"wv{ki}",
                                    name=f"wv{ki}")
                       for ki in range(KCH)]
                wkt = [wp_pool.tile([128, DH], BF16, tag=f"wk{ki}",
                                    name=f"wk{ki}")
                       for ki in range(KCH)]
                wqt = [wp_pool.tile([128, DH], BF16, tag=f"wq{ki}",
                                    name=f"wq{ki}")
                       for ki in range(KCH)]
                xkt = [xkq_pool.tile([128, S], BF16, tag=f"xk{ki}",
                                     name=f"xk{ki}")
                       for ki in range(KCH)]
                xqt = [xkq_pool.tile([128, S], BF16, tag=f"xq{ki}",
                                     name=f"xq{ki}")
                       for ki in range(KCH)]
                # weights on the scalar queue (its engine is idle early)
                for ki in range(KCH):
                    nc.scalar.dma_start(out=wvt[ki][:, :],
                                        in_=wv[128 * ki:128 * (ki + 1), :])
                for ki in range(KCH):
                    nc.scalar.dma_start(out=wkt[ki][:, :],
                                        in_=wk[128 * ki:128 * (ki + 1), :])
                    nc.scalar.dma_start(out=wqt[ki][:, :],
                                        in_=wq[128 * ki:128 * (ki + 1), :])
                # xv in [128, 512] column-chunks on gpsimd + vector queues so
                # the first V-proj st-group can start after ~2 MB, not 4.2.
                xv_ctx = tc.tile_pool(name="xv", bufs=1)
                xv_pool = xv_ctx.__enter__()
                # xvc[half][cs][ki] covers xv[:, 1024*half + 512*cs ...].
                # Tags are shared between the halves (bufs=1): the half-1 DMA
                # reuses half-0's slot once the V-proj has consumed it, halving
                # the pool footprint; the wait hides under half-0's compute.
                xvc = [[[None] * KCH for _ in range(2)] for _ in range(2)]
                for half in range(2):
                    for cs in range(2):
                        c0 = 1024 * half + 512 * cs
                        for ki in range(KCH):
                            t = xv_pool.tile([128, 512], BF16,
                                             tag=f"xv{cs}_{ki}",
                                             name=f"xv{half}_{cs}_{ki}")
                            xvc[half][cs][ki] = t
                            nc.gpsimd.dma_start(
                                out=t[:, :],
                                in_=xv[128 * ki:128 * (ki + 1), c0:c0 + 512])
                for ki in range(KCH):
                    nc.sync.dma_start(out=xkt[ki][:, :],
                                      in_=xk[128 * ki:128 * (ki + 1), :])
                for ki in range(KCH):
                    nc.sync.dma_start(out=xqt[ki][:, :],
                                      in_=xq[128 * ki:128 * (ki + 1), :])
                # Wo on the vector queue behind the xv chunks; ready long
                # before the O-projection needs it.
                wot = [wo_pool.tile([128, D], BF16, tag=f"wo{c}",
                                    name=f"wot{c}")
                       for c in range(DCH)]
                for c in range(DCH):
                    nc.scalar.dma_start(
                        out=wot[c][:, :], in_=wo[128 * c:128 * (c + 1), :])

                # ---- V projection ----
                for half in range(2):
                    for st in range(8):
                        cs, so = st // 4, st % 4
                        ps = psA.tile([128, DH], F32, tag="psA", name="psAv_")
                        for ki in range(KCH):
                            nc.tensor.matmul(
                                ps[:, :],
                                xvc[half][cs][ki][:, 128 * so:128 * (so + 1)],
                                wvt[ki][:, :],
                                start=(ki == 0),
                                stop=(ki == KCH - 1),
                            )
                        vdst = VN[8 * half + st][:, :].rearrange(
                            "p (h c) -> p h c", c=HD + 1)[:, :, :HD]
                        nc.vector.tensor_add(
                            vdst,
                            ps[:, :].rearrange("p (h c) -> p h c", c=HD),
                            bvb[:, :].rearrange("p (h c) -> p h c", c=HD),
                        )
                xv_ctx.__exit__(None, None, None)

                # ---- per pair: K/Q projections then attention ----
                atp_tiles = []
                for p in range(DCH):
                    for wt, xt, dest, bias in ((wkt, xkt, KT, bks),
                                               (wqt, xqt, QT, bqs)):
                        for sc in range(NQT):
                            ps = psA.tile([128, 512], F32, tag="psA",
                                          name="psA_")
                            for ki in range(KCH):
                                nc.tensor.matmul(
                                    ps[:, :],
                                    wt[ki][:, 128 * p:128 * (p + 1)],
                                    xt[ki][:, 512 * sc:512 * (sc + 1)],
                                    start=(ki == 0),
                                    stop=(ki == KCH - 1),
                                )
                            nc.vector.tensor_scalar_add(
                                dest[p][:, 512 * sc:512 * (sc + 1)],
                                ps[:, :],
                                bias[:, p:p + 1],
                            )

                    atp = atp_pool.tile([128, S], BF16, tag="atp",
                                        name="atp_")
                    atp_tiles.append(atp)
                    for j in range(NQT):
                        q0 = 512 * j
                        nk = 4 * (j + 1)  # valid k-tiles (causal)
                        pv = [psV.tile([HD + 1, 512], F32, tag=f"pv{h}",
                                       name=f"pv{h}_")
                              for h in range(2)]
                        for g in range(0, nk, GRP):
                            pss = [psS.tile([128, 512 * GRP], F32, tag="psS",
                                            name="psS_")
                                   for _ in range(2)]
                            for m in range(GRP):
                                k = g + m
                                # crossing k-tile i only covers query columns
                                # >= 128i of this 512-q block
                                i = k - 4 * j
                                v0 = 128 * i if 0 <= i <= 3 else 0
                                for h in range(2):
                                    r0 = 64 * h
                                    nc.tensor.matmul(
                                        pss[h][:, 512 * m + v0:
                                               512 * (m + 1)],
                                        KT[p][r0:r0 + 64,
                                              128 * k:128 * (k + 1)],
                                        QT[p][r0:r0 + 64,
                                              q0 + v0:q0 + 512],
                                        start=True,
                                        stop=True,
                                    )
                            prt = [pr_pool.tile([128, 512 * GRP], BF16,
                                                tag="pr", name="pr_")
                                   for _ in range(2)]
                            for h in range(2):
                                nc.scalar.activation(
                                    prt[h][:, :], pss[h][:, :], EXP,
                                    scale=0.125)
                            # causal mask on diagonal-crossing k-tiles
                            # (valid columns only)
                            for h in range(2):
                                for m in range(GRP):
                                    i = g + m - 4 * j
                                    if 0 <= i <= 3:
                                        nc.vector.tensor_mul(
                                            prt[h][:, 512 * m + 128 * i:
                                                   512 * (m + 1)],
                                            prt[h][:, 512 * m + 128 * i:
                                                   512 * (m + 1)],
                                            mask[:, 384:896 - 128 * i],
                                        )
                            for m in range(GRP):
                                k = g + m
                                i = k - 4 * j
                                v0 = 128 * i if 0 <= i <= 3 else 0
                                for h in range(2):
                                    hl = 2 * p + h
                                    nc.tensor.matmul(
                                        pv[h][:, v0:],
                                        VN[k][:, 65 * hl:65 * hl + 65],
                                        prt[h][:, 512 * m + v0:
                                               512 * (m + 1)],
                                        start=(k == 0),
                                        stop=(k == nk - 1),
                                        skip_group_check=True,
                                    )
                        # drain: numerator -> atp; reciprocal of the
                        # denominator row -> DRAM bounce; broadcast back and
                        # normalize on gpsimd, all pipelined per j-tile.
                        for h in range(2):
                            hl = 2 * p + h
                            nc.vector.tensor_copy(
                                atp[64 * h:64 * h + 64, q0:q0 + 512],
                                pv[h][:HD, :],
                            )
                            rc = nrm_pool.tile([1, 512], F32, tag=f"rc{h}",
                                               name=f"rc{h}_")
                            nc.vector.reciprocal(rc[:, :],
                                                 pv[h][HD:HD + 1, :])
                            nc.sync.dma_start(
                                out=rden[hl:hl + 1, q0:q0 + 512],
                                in_=rc[:, :],
                            )
                        bct = nrm_pool.tile([128, 512], F32, tag="bct",
                                            name="bct_")
                        for h in range(2):
                            nc.sync.dma_start(
                                out=bct[64 * h:64 * h + 64, :],
                                in_=rden[2 * p + h:2 * p + h + 1,
                                         q0:q0 + 512]
                                .partition_broadcast(64),
                            )
                        nc.gpsimd.tensor_mul(
                            atp[:, q0:q0 + 512],
                            atp[:, q0:q0 + 512],
                            bct[:, :],
                        )

            # ---------------- Output projection ----------------
                for qt in range(NKT):  # 16 q tiles of 128
                    q0 = 128 * qt
                    for n in range(2):
                        ps = psA.tile([128, 512], F32, tag="psA", name="psO_")
                        for c in range(DCH):
                            nc.tensor.matmul(
                                ps[:, :],
                                atp_tiles[c][:, q0:q0 + 128],
                                wot[c][:, 512 * n:512 * (n + 1)],
                                start=(c == 0),
                                stop=(c == DCH - 1),
                            )
                        ot = ob_pool.tile([128, 512], F32, tag="ob",
                                          name="ob_")
                        nc.vector.tensor_copy(ot[:, :], ps[:, :])
                        nc.sync.dma_start(
                            out=out[q0:q0 + 128, 512 * n:512 * (n + 1)],
                            in_=ot[:, :])
            ob_ctx.__exit__(None, None, None)
            wo_ctx.__exit__(None, None, None)
            atp_ctx.__exit__(None, None, None)

    nc.compile()
    return nc


def kernel(query, key, value, Wq, bq, Wk, bk, Wv, bv, Wo, bo, **trace_kwargs):
    from concourse.bass_utils import run_bass_kernel_spmd

    global _compiled
    if _compiled is None:
        _compiled = _build()
    nc = _compiled

    import ml_dtypes

    BF = ml_dtypes.bfloat16
    query = np.asarray(query, np.float32)
    key = np.asarray(key, np.float32)
    value = np.asarray(value, np.float32)
    Wq, Wk, Wv, Wo = (np.asarray(w, np.float32) for w in (Wq, Wk, Wv, Wo))
    bq, bk, bv, bo = (np.asarray(b_, np.float32) for b_ in (bq, bk, bv, bo))

    xqT = [np.ascontiguousarray(query[b].T).astype(BF) for b in range(B)]
    xkT = [np.ascontiguousarray(key[b].T).astype(BF) for b in range(B)]
    xvT = [np.ascontiguousarray(value[b].T).astype(BF) for b in range(B)]
    shard = []
    for t in range(TP):
        cs = slice(DH * t, DH * (t + 1))
        shard.append({
            "wq": np.ascontiguousarray(Wq[:, cs]).astype(BF),
            "wk": np.ascontiguousarray(Wk[:, cs]).astype(BF),
            "wv": np.ascontiguousarray(Wv[:, cs]).astype(BF),
            "wo": np.ascontiguousarray(Wo[cs, :]).astype(BF),
            "bq_c": np.ascontiguousarray(bq[cs].reshape(DCH, 128).T),
            "bk_c": np.ascontiguousarray(bk[cs].reshape(DCH, 128).T),
            "bv_b": np.ascontiguousarray(
                np.broadcast_to(bv[cs], (128, DH))),
        })

    in_maps = []
    for c in range(8):
        b, t = c // TP, c % TP
        m = {"xq": xqT[b], "xk": xkT[b], "xv": xvT[b]}
        m.update(shard[t])
        in_maps.append(m)

    res = run_bass_kernel_spmd(nc, in_maps, core_ids=list(range(8)),
                               **trace_kwargs)
    outp = np.empty((B, S, D), np.float32)
    for b in range(B):
        outp[b] = res.results[TP * b]["out"] + res.results[TP * b + 1]["out"] + bo
    if trace_kwargs:
        kernel.last_results = res
    return outp
